# revision 72
# baseline (speedup 1.0000x reference)
"""Trainium2 Bass kernel for the Chambolle-Pock-style primal/dual stencil loop.

Math (per image, H=W=1024, EPS=0.5, TAU=0.5, 10 iterations):
    u = sigmoid(o/EPS); q = 0
    repeat 10x:
        q  = relu(q - TAU*(vf1*Dy(u) + vf0*Dx(u)))    # forward diffs, zero pad
        Tq = BDy(vf1*q) + BDx(vf0*q)                  # backward diffs, zero pad
        u  = sigmoid((o - Tq)/EPS)
    return (o - Tq)/EPS

Rescaled (qh = 2*sqrt(2)*q, g = vf/sqrt(2), o2 = 2*o, t = tanh(s/2) with
s = 2(o - Tq); u-padding 0 becomes t-padding -1):
    dual:   qh = relu(qh + gs*t - g1*t(y+1) - g0*t(x+1))       A, -B, -C
    primal: s  = o2 - D + F(x-1) + H(y-1),  D = gs*qh, F = g0*qh, H = g1*qh
    t = tanh(s/2); output = s of the last iteration.

All state/products fp16 (bf16 fails the 2e-2 gate); primal/dual sums
accumulate in fp32 PSUM via +-identity matmuls.

Engine balance per iteration (period ~25us):
  - DVE  (~24.6us): A = gs*t, B = g1*t(y+1v) products, quarter q0's dual
    chain (plane-split), D = gs*qh, H = g1*qh, F' for quarters q2/q3.
  - Pool (~22.9us): C = g0*t(x+1v) and F' of q1/q0p1 as plain tensor_mul
    (GPSIMD is the only engine besides DVE that can do elementwise
    two-tensor products; scalar_tensor_tensor is NOT a legal Pool opcode
    on real TRN2 even though the cost model prices it).
  - PE   (~23.9us, 112 matmuls): dual sums for q1,q2,q3 (qh,A,-B,-C) and
    primal sums (o2,-D,F'(x-1) via column-offset PSUM writes, H(y-1) via
    plane views), one 512-col matmul per PSUM half-bank-pair, 1-plane
    PSUM groups (2 banks) x 4 rotating buffers so PE never waits long
    for an Act exit.  Group heads (qh / o2) have no in-iteration deps.
  - Act  (~16.6us): per-plane relu / tanh exits (PSUM->SBUF fp16).

Cross-iteration software pipelining *of emission order*: the dual
products of iteration i+1 are emitted interleaved into iteration i's
primal section, so the in-order DVE/Pool queues always have ready work
while quarter q0's serial tail (chain -> relu -> products -> PE -> tanh)
drains.  Quarter order (1,2,3,0); q0's dual chain is plane-split with
plane 1 first because the H wrap row (H(8p+1), feeding q1's primal PE)
depends on it.

Shifts: y+1 reads are plane views; the plane-8 boundary row (t(8p+8))
moves by a small SBUF->SBUF DMA each iteration (t(1024)=-1 pinned in
partition 127); B of q3 is plane-split so only its plane-7 half waits on
that DMA's ~4us latency.  x+1 reads use a -1 guard column at x=W.
H(y-1) uses plane views with a plane-0 boundary DMA.  F(x-1) is
expressed by writing PSUM at column offset 1 (s(x=0) gets no F term).

Loads: only o2, g0, g1 (6MB fp16) move over HBM; gs = g0 + g1 is derived
on-chip (one-time DVE cost hidden under the load-bound first iteration).
DMA transfers serialize in hardware, so the load order follows first-use
order (q1 fields, ident, q3/q0 o2, q0 fields, q2, q3).

Sharding: pure data parallel, one image per NeuronCore (B=8 over 8 cores),
g-fields broadcast.
"""

import numpy as np

import concourse.bacc as bacc
import concourse.mybir as mybir
from concourse.tile import TileContext
from concourse import bass_utils

F16 = mybir.dt.float16
F32 = mybir.dt.float32
AF = mybir.ActivationFunctionType
AL = mybir.AluOpType

B, H, W = 8, 1024, 1024
P = 128          # SBUF partitions
NP = H // P      # planes per partition = 8
WG = W + 2       # t-plane width incl. guard column (even, keeps 4B align)
MAXITER = 10
BK = 512         # PSUM bank = 512 fp32 = one matmul's max output

Q1, Q2, Q3, Q0 = (2, 4), (4, 6), (6, 8), (0, 2)
QORD = (Q1, Q2, Q3, Q0)

_CACHE = {}
LAST_RESULTS = None  # BassKernelResults of the most recent run (for test.py)


def _build():
    nc = bacc.Bacc("TRN2", target_bir_lowering=False, debug=False)

    o2_d = nc.dram_tensor("o2", [H, W], F16, kind="ExternalInput").ap()
    g0_d = nc.dram_tensor("g0", [H, W], F16, kind="ExternalInput").ap()
    g1_d = nc.dram_tensor("g1", [H, W], F16, kind="ExternalInput").ap()
    id_d = nc.dram_tensor("ident", [P, P], F16, kind="ExternalInput").ap()
    nid_d = nc.dram_tensor("nident", [P, P], F16, kind="ExternalInput").ap()
    mone_d = nc.dram_tensor("mone", [1, W], F16, kind="ExternalInput").ap()
    out_d = nc.dram_tensor("out", [H, W], F16, kind="ExternalOutput").ap()

    # (H, W) -> (p, i, x) with y = 8*p + i
    def vu(ap):
        return ap.rearrange("(p i) x -> p i x", i=NP)

    v = nc.vector
    gp = nc.gpsimd
    act = nc.scalar
    pe = nc.tensor

    with TileContext(nc) as tc:
        with tc.tile_pool(name="main", bufs=1) as pool:
            o2t = pool.tile([P, NP, W], F16)
            g0t = pool.tile([P, NP, W], F16)
            g1t = pool.tile([P, NP, W], F16)
            gst = pool.tile([P, NP, W], F16)
            # t state: planes 0..7, plane 8 = y+1 boundary row (t(8p+8)),
            # col W = -1 guard for x+1 reads
            sut = pool.tile([P, NP + 1, WG], F16)
            qht = pool.tile([P, NP, W], F16)
            tA = pool.tile([P, NP, W], F16)   # A = gs*t, then D = gs*qh
            tB = pool.tile([P, NP, W], F16)   # B = g1*t(y+1)
            tC = pool.tile([P, NP, W], F16)   # C = g0*t(x+1), then F' = g0*qh
            # H = g1*qh at planes 1..8; plane 0 = H(8p-1) boundary
            # (partition 0: zero pad)
            tH = pool.tile([P, NP + 1, W], F16)
            idt = pool.tile([P, P], F16)
            nidt = pool.tile([P, P], F16)

            # --- guard/pad init (before anything else) ---
            v.memset(sut[:, :, W:WG], -1.0)          # x+1 guard columns
            # t(1024) = -1 pad row sits alone in partition 127; engines
            # can't address a partition range starting there, so DMA it
            nc.sync.dma_start(out=sut[P - 1 : P, NP : NP + 1, 0:W], in_=mone_d)
            v.memset(tH[0:1, 0, :], 0.0)             # H(-1) pad row = 0

            # --- loads, in first-use order (DMA transfers serialize) ---
            def ld(tile, dram, lo, hi):
                nc.sync.dma_start(out=tile[:, lo:hi, :], in_=vu(dram)[:, lo:hi, :])
            def dma_tshift():
                # t plane 8 (row 8p+8) <- partition p+1 plane 0
                nc.sync.dma_start(
                    out=sut[0 : P - 1, NP : NP + 1, 0:W],
                    in_=sut[1:P, 0:1, 0:W],
                )

            def dma_hshift():
                # tH[p, 0] = H(8p-1) = tH[p-1, 8]; partition 0 stays 0
                nc.sync.dma_start(
                    out=tH[1:P, 0:1, :], in_=tH[0 : P - 1, NP : NP + 1, :]
                )

            # Loads in first-use order.  q0's g-fields go LAST so the
            # t-shift transfer (which only needs tanh of o2-q0) can slot
            # into the single-file DMA queue ahead of them.
            ld(o2t, o2_d, *Q1)
            ld(o2t, o2_d, *Q2)
            ld(g0t, g0_d, *Q1)
            ld(g1t, g1_d, *Q1)
            nc.sync.dma_start(out=idt[:, :], in_=id_d)
            nc.sync.dma_start(out=nidt[:, :], in_=nid_d)
            ld(o2t, o2_d, *Q3)
            ld(o2t, o2_d, *Q0)
            ld(g0t, g0_d, *Q2)
            ld(g1t, g1_d, *Q2)
            ld(g0t, g0_d, *Q3)
            ld(g1t, g1_d, *Q3)

            # --- t init ---
            for lo, hi in QORD:
                act.activation(
                    sut[:, lo:hi, 0:W], o2t[:, lo:hi, :], AF.Tanh, scale=0.5
                )
            dma_tshift()
            ld(g0t, g0_d, *Q0)
            ld(g1t, g1_d, *Q0)

            def dual_products():
                # Iteration 0 only (steady iterations emit their products
                # inline).  Per-quarter [derive gs, A, B] clusters in load
                # order so no derive head-of-line-blocks the DVE queue on a
                # later quarter's field loads; q0 (fields last) closes.
                for lo, hi in (Q1, Q2, Q3, Q0):
                    v.tensor_add(
                        gst[:, lo:hi, :], g0t[:, lo:hi, :], g1t[:, lo:hi, :]
                    )
                    v.tensor_mul(
                        tA[:, lo:hi, :], gst[:, lo:hi, :], sut[:, lo:hi, 0:W]
                    )
                    if lo == 6:
                        v.tensor_mul(
                            tB[:, 6:7, :], g1t[:, 6:7, :], sut[:, 7:8, 0:W]
                        )
                        v.tensor_mul(
                            tB[:, 7:8, :], g1t[:, 7:8, :], sut[:, 8:9, 0:W]
                        )
                    else:
                        v.tensor_mul(
                            tB[:, lo:hi, :], g1t[:, lo:hi, :],
                            sut[:, lo + 1 : hi + 1, 0:W],
                        )
                for lo, hi in (Q1, Q2, Q3, Q0):
                    gp.tensor_mul(
                        tC[:, lo:hi, :], g0t[:, lo:hi, :],
                        sut[:, lo:hi, 1 : W + 1],
                    )

            with tc.tile_pool(name="ps", bufs=4, space="PSUM") as pp:

                def alloc_ps():
                    # 1-plane groups (2 PSUM banks) x 4 rotating buffers
                    return pp.tile([P, 1, W], F32, name="ps")

                def dual_pe_q(lo, hi, first=False):
                    for p in (lo, lo + 1):
                        ps = alloc_ps()
                        srcs = [] if first else [(idt, qht)]
                        srcs += [(idt, tA), (nidt, tB), (nidt, tC)]
                        for i_s, (w, tile) in enumerate(srcs):
                            for c in (0, BK):
                                pe.matmul(
                                    ps[:, 0, c : c + BK], w[:, :],
                                    tile[:, p, c : c + BK],
                                    start=(i_s == 0),
                                    stop=(i_s == len(srcs) - 1),
                                )
                        act.activation(
                            qht[:, p : p + 1, :], ps[:, :, :], AF.Relu
                        )

                def q0_chain(p, first):
                    q_ = qht[:, p : p + 1, :]
                    if first:
                        v.tensor_sub(q_, tA[:, p : p + 1, :], tB[:, p : p + 1, :])
                    else:
                        v.tensor_add(q_, q_, tA[:, p : p + 1, :])
                        v.tensor_sub(q_, q_, tB[:, p : p + 1, :])
                    v.tensor_sub(q_, q_, tC[:, p : p + 1, :])
                    if p == 1:
                        # plane 1's relu on DVE (tensor_scalar, 4 elem/cyc):
                        # the H wrap product follows it in the in-order DVE
                        # queue and must not stall on an Act round-trip
                        v.tensor_scalar(q_, q_, 0.0, None, op0=AL.max)
                    else:
                        act.activation(q_, q_, AF.Relu)

                def primal_pe(lo, hi, last, no_d=False):
                    # no_d: s = o2 - F' - H + F'(x-1) + H(y-1), absorbing
                    # D = F' + H into two extra matmuls instead of a DVE
                    # product — used for q0, whose D sat on the loop-carried
                    # relu-p0 -> tanh-q0 critical cycle
                    for p in (lo, lo + 1):
                        ps = alloc_ps()
                        for c in (0, BK):
                            pe.matmul(
                                ps[:, 0, c : c + BK], idt[:, :],
                                o2t[:, p, c : c + BK],
                                start=True, stop=False,
                            )
                        if no_d:
                            for c in (0, BK):
                                pe.matmul(
                                    ps[:, 0, c : c + BK], nidt[:, :],
                                    tC[:, p, c : c + BK],
                                    start=False, stop=False,
                                )
                            for c in (0, BK):
                                pe.matmul(
                                    ps[:, 0, c : c + BK], nidt[:, :],
                                    tH[:, p + 1, c : c + BK],
                                    start=False, stop=False,
                                )
                        else:
                            for c in (0, BK):
                                pe.matmul(
                                    ps[:, 0, c : c + BK], nidt[:, :],
                                    tA[:, p, c : c + BK],
                                    start=False, stop=False,
                                )
                        pe.matmul(
                            ps[:, 0, 1:BK], idt[:, :],
                            tC[:, p, 0 : BK - 1],
                            start=False, stop=False,
                        )
                        pe.matmul(
                            ps[:, 0, BK:W], idt[:, :],
                            tC[:, p, BK - 1 : W - 1],
                            start=False, stop=False,
                        )
                        for c in (0, BK):
                            pe.matmul(
                                ps[:, 0, c : c + BK], idt[:, :],
                                tH[:, p, c : c + BK],
                                start=False, stop=True,
                            )
                        if not last:
                            act.activation(
                                sut[:, p : p + 1, 0:W], ps[:, :, :],
                                AF.Tanh, scale=0.5,
                            )
                        else:
                            act.activation(
                                sut[:, p : p + 1, 0:W], ps[:, :, :], AF.Copy
                            )
                            nc.sync.dma_start(
                                out=vu(out_d)[:, p : p + 1, :],
                                in_=sut[:, p : p + 1, 0:W],
                            )

                # iteration 0's dual products stand alone (its t-shift was
                # emitted up in the load section, ahead of q0's g-fields in
                # the DMA queue)
                dual_products()

                for it in range(MAXITER):
                    first = it == 0
                    last = it == MAXITER - 1

                    # --- dual sums: PE (q1,q2,q3) + q0 DVE chains.
                    # In steady state q1/q2's PE groups were already
                    # emitted during the previous iteration's tail (they
                    # overlap the q0 serial tail on PE); iteration 0 emits
                    # them here ---
                    if first:
                        dual_pe_q(*Q1, first=True)
                        dual_pe_q(*Q2, first=True)
                    dual_pe_q(*Q3, first=first)
                    q0_chain(1, first)
                    v.tensor_mul(tH[:, 2:3, :], g1t[:, 1:2, :], qht[:, 1:2, :])
                    # iteration 0: F'0p1's chain-relu dep would head-of-line
                    # block Pool's queue ahead of the C' products the hoisted
                    # dual-PE groups need; spill it to DVE once
                    (v if first else gp).tensor_mul(
                        tC[:, 1:2, :], g0t[:, 1:2, :], qht[:, 1:2, :]
                    )
                    q0_chain(0, first)

                    # --- primal (products, PE, exits), interleaved with
                    # the NEXT iteration's dual products (emitted as soon
                    # as their t planes exit) so the in-order DVE/Pool/PE
                    # queues never drain on q0's serial tail ---
                    # q1's H/D split per plane: plane 2's products start on
                    # relu-q1p2 without waiting for plane 3's relu
                    v.tensor_mul(tH[:, 3:4, :], g1t[:, 2:3, :], qht[:, 2:3, :])
                    v.tensor_mul(tA[:, 2:3, :], gst[:, 2:3, :], qht[:, 2:3, :])
                    v.tensor_mul(tH[:, 4:5, :], g1t[:, 3:4, :], qht[:, 3:4, :])
                    v.tensor_mul(tA[:, 3:4, :], gst[:, 3:4, :], qht[:, 3:4, :])
                    # iteration 0: Pool is swamped (its own C's serialized
                    # behind the HBM loads plus iteration 1's C's), so spill
                    # its share of products to DVE once
                    (v if first else gp).tensor_mul(
                        tC[:, 2:4, :], g0t[:, 2:4, :], qht[:, 2:4, :]
                    )
                    primal_pe(*Q1, last)

                    v.tensor_mul(tH[:, 5:7, :], g1t[:, 4:6, :], qht[:, 4:6, :])
                    v.tensor_mul(tA[:, 4:6, :], gst[:, 4:6, :], qht[:, 4:6, :])
                    v.tensor_mul(tC[:, 4:6, :], g0t[:, 4:6, :], qht[:, 4:6, :])
                    primal_pe(*Q2, last)
                    if not last:
                        # next iter, quarter q1 (t planes 2..4 have exited;
                        # one quarter behind the PE readers of tA/tC so the
                        # WAR waits don't stall the in-order DVE queue).
                        # B' first: it has no pending readers, buying PE
                        # time to clear its tA reads before A' lands
                        v.tensor_mul(
                            tB[:, 2:4, :], g1t[:, 2:4, :], sut[:, 3:5, 0:W]
                        )
                        v.tensor_mul(
                            tA[:, 2:4, :], gst[:, 2:4, :], sut[:, 2:4, 0:W]
                        )
                        gp.tensor_mul(
                            tC[:, 2:4, :], g0t[:, 2:4, :], sut[:, 2:4, 1 : W + 1]
                        )

                    v.tensor_mul(tH[:, 7:9, :], g1t[:, 6:8, :], qht[:, 6:8, :])
                    v.tensor_mul(tA[:, 6:8, :], gst[:, 6:8, :], qht[:, 6:8, :])
                    v.tensor_mul(tC[:, 6:8, :], g0t[:, 6:8, :], qht[:, 6:8, :])
                    dma_hshift()
                    primal_pe(*Q3, last)
                    if not last:
                        # next iter, quarters q2/q3 heads (B' products first
                        # for the same WAR reason)
                        v.tensor_mul(
                            tB[:, 4:6, :], g1t[:, 4:6, :], sut[:, 5:7, 0:W]
                        )
                        v.tensor_mul(tB[:, 6:7, :], g1t[:, 6:7, :], sut[:, 7:8, 0:W])
                        v.tensor_mul(
                            tA[:, 4:6, :], gst[:, 4:6, :], sut[:, 4:6, 0:W]
                        )
                        v.tensor_mul(
                            tA[:, 6:8, :], gst[:, 6:8, :], sut[:, 6:8, 0:W]
                        )
                        if first:
                            # C2' spill split: the 1-plane Pool half may hold
                            # the d'(q2) deadline the 2-plane version blew
                            gp.tensor_mul(
                                tC[:, 4:5, :], g0t[:, 4:5, :], sut[:, 4:5, 1 : W + 1]
                            )
                            v.tensor_mul(
                                tC[:, 5:6, :], g0t[:, 5:6, :], sut[:, 5:6, 1 : W + 1]
                            )
                        else:
                            gp.tensor_mul(
                                tC[:, 4:6, :], g0t[:, 4:6, :], sut[:, 4:6, 1 : W + 1]
                            )

                        # next iteration's q1/q2 dual-PE groups: they keep
                        # PE busy while this iteration's q0 serial tail
                        # (chain -> products -> PE -> tanh) drains
                        dual_pe_q(*Q1)
                        dual_pe_q(*Q2)

                    v.tensor_mul(tH[:, 1:2, :], g1t[:, 0:1, :], qht[:, 0:1, :])
                    v.tensor_mul(tA[:, 0:2, :], gst[:, 0:2, :], qht[:, 0:2, :])
                    v.tensor_mul(tC[:, 0:1, :], g0t[:, 0:1, :], qht[:, 0:1, :])
                    primal_pe(*Q0, last)
                    if not last:
                        dma_tshift()
                        v.tensor_mul(
                            tA[:, 0:2, :], gst[:, 0:2, :], sut[:, 0:2, 0:W]
                        )
                        v.tensor_mul(
                            tB[:, 0:2, :], g1t[:, 0:2, :], sut[:, 1:3, 0:W]
                        )
                        gp.tensor_mul(
                            tC[:, 6:8, :], g0t[:, 6:8, :], sut[:, 6:8, 1 : W + 1]
                        )
                        gp.tensor_mul(
                            tC[:, 0:2, :], g0t[:, 0:2, :], sut[:, 0:2, 1 : W + 1]
                        )
                        v.tensor_mul(tB[:, 7:8, :], g1t[:, 7:8, :], sut[:, 8:9, 0:W])

    nc.compile()
    return nc


def kernel(o, vector_field, nabla_w, div_w):
    global LAST_RESULTS
    if "nc" not in _CACHE:
        _CACHE["nc"] = _build()
    nc = _CACHE["nc"]

    o2 = (2.0 * np.asarray(o, dtype=np.float32)[:, 0]).astype(np.float16)
    vf = np.asarray(vector_field, dtype=np.float32)
    s = np.float32(1.0 / np.sqrt(2.0))
    g0 = (vf[:, :, 0] * s).astype(np.float16)
    g1 = (vf[:, :, 1] * s).astype(np.float16)
    ident = np.eye(P, dtype=np.float16)

    mone = np.full((1, W), -1.0, dtype=np.float16)
    in_maps = [
        {
            "o2": np.ascontiguousarray(o2[b]),
            "g0": g0,
            "g1": g1,
            "ident": ident,
            "nident": -ident,
            "mone": mone,
        }
        for b in range(B)
    ]
    res = bass_utils.run_bass_kernel_spmd(nc, in_maps, core_ids=list(range(B)))
    LAST_RESULTS = res
    return np.stack([r["out"] for r in res.results]).astype(np.float32)
